# revision 15
# baseline (speedup 1.0000x reference)
"""CenterLoss on 8 Trainium2 NeuronCores — v8: sort-by-class, hybrid
TensorE + SWDGE datapaths.

mean_i clip(||features_i - centers[labels_i,-1]||^2, 1e-12, 1e12) for
features [16384, 512] f32, labels [16384, 2] int, centers [10000, 512] f32.

HOST sorts rows by class and shards them (the mean is permutation-
invariant); all inputs are cast to fp8 e4m3 (2.8e-4 rel err vs the 2e-2
gate). Per core, 16 tiles of 128 rows each are processed by TWO pipelines
sized to the engines' measured speeds:

  - tiles 0-11 (TensorE path): each sorted tile spans a contiguous class
    window of <= 128 ids, so one fp8 DoubleRow matmul per tile both
    selects the centers and subtracts:
        PSUM = I.T @ f + G_t.T @ f8(-centers)[a_t:a_t+128] = f - c_y
    (G_t = host-built one-hot; payload packed per tile as [f|I],[win|G]
    in one DRAM tensor, DMAed in sequenced waves - SDMA drains concurrent
    queues round-robin at equal rates, so waves go out at most ~3 in
    flight with the first wave small to start TensorE early).
    ACT squares the PSUM diffs (Square+accum_out) in bank-range ops; the
    PE runs at its low p-state (~0.66us/tile incl LDWEIGHTS), which is
    why it only gets 12 tiles.
  - tiles 12-15 (SWDGE path, otherwise-idle engines): GpSimd issues one
    indirect DMA per tile (builtin, no ucode library; ~1.3us/tile of Q7
    descriptor-gen), gathering the 128 matching center rows; DVE
    subtracts (f8->bf16) and squares (bf16 self-multiply, 2x mode) with
    accum_out.

The clamp is a no-op (d2 ~ 680 >> 1e-12) and is dropped. ones^T @ acc on
TensorE folds partitions, reduce_sum -> [1,1], host sums 8 partials / N.
"""

import sys

if "/opt/trn_rl_repo" not in sys.path:
    sys.path.insert(0, "/opt/trn_rl_repo")

import numpy as np

N, D, C = 16384, 512, 10000
N_CORES = 8
NS = N // N_CORES  # 2048 rows per core
P = 128
NT = NS // P  # 16 tiles of 128 rows per core
NTE = 12  # tiles on the TensorE path
NGT = NT - NTE  # tiles on the SWDGE gather path
W = D + P  # 640: per-half payload (512 f/win + 128 I/G)
WAVES = [1, 2, 3, 3, 3]  # TE-path tiles per DMA wave (ascending)
assert sum(WAVES) == NTE
# ACT square ops over TE tiles: (tile_lo, tile_hi); PSUM bank = tile % 8
SQ_OPS = [(0, 2), (2, 5), (5, 8), (8, 11), (11, 12)]

_cache = {}


def _build():
    from contextlib import ExitStack

    from concourse import bacc, bass, mybir

    f8 = mybir.dt.float8e4

    nc = bacc.Bacc("TRN2", target_bir_lowering=False, debug=False)
    # TE path: packed per tile and half: [f_t | I], [win_t | G_t]
    src = nc.dram_tensor("src", [P, NTE * 2 * W], f8, kind="ExternalInput")
    # gather path: features + class ids of tiles 12-15, negated f8 centers
    fg = nc.dram_tensor("fg", [P, NGT * D], f8, kind="ExternalInput")
    idx = nc.dram_tensor("idx", [P, NGT], mybir.dt.int32, kind="ExternalInput")
    cent = nc.dram_tensor("cent", [C, D], f8, kind="ExternalInput")  # negated
    out = nc.dram_tensor("out", [1, 1], mybir.dt.float32, kind="ExternalOutput")

    NAC = len(SQ_OPS) + NGT  # acc columns: ACT ops then gather tiles

    with ExitStack() as ctx:
        mega = ctx.enter_context(nc.sbuf_tensor([P, NTE, 2, W], f8))
        fgb = ctx.enter_context(nc.sbuf_tensor([P, NGT, D], f8))
        cbuf = ctx.enter_context(nc.sbuf_tensor([P, NGT, D], f8))
        dbuf = ctx.enter_context(nc.sbuf_tensor([P, NGT, D], mybir.dt.bfloat16))
        idx_sb = ctx.enter_context(nc.sbuf_tensor([P, NGT], mybir.dt.int32))
        wscr = ctx.enter_context(nc.sbuf_tensor([P, 2, D], f8))
        acc = ctx.enter_context(nc.sbuf_tensor([P, NAC], mybir.dt.float32))
        ones = ctx.enter_context(nc.sbuf_tensor([P, 1], mybir.dt.float32))
        scr = ctx.enter_context(nc.sbuf_tensor([P, 1], mybir.dt.float32))
        red = ctx.enter_context(nc.sbuf_tensor([1, 1], mybir.dt.float32))
        ps = ctx.enter_context(nc.psum_tensor([P, 8, D], mybir.dt.float32))
        s_v = [ctx.enter_context(nc.semaphore(f"s_v{k}")) for k in range(len(WAVES))]
        s_ix = ctx.enter_context(nc.semaphore("s_ix"))
        s_fb = ctx.enter_context(nc.semaphore("s_fb"))
        s_g = [ctx.enter_context(nc.semaphore(f"s_g{j}")) for j in range(NGT)]
        s_ones = ctx.enter_context(nc.semaphore("s_ones"))
        s_scr = ctx.enter_context(nc.semaphore("s_scr"))
        s_wscr = ctx.enter_context(nc.semaphore("s_wscr"))
        s_d = ctx.enter_context(nc.semaphore("s_d"))
        s_sub = ctx.enter_context(nc.semaphore("s_sub"))
        s_sqa = ctx.enter_context(nc.semaphore("s_sqa"))
        s_sqd = ctx.enter_context(nc.semaphore("s_sqd"))
        s_mm = ctx.enter_context(nc.semaphore("s_mm"))
        s_red = ctx.enter_context(nc.semaphore("s_red"))
        s_od = ctx.enter_context(nc.semaphore("s_od"))
        block = ctx.enter_context(nc.Block(no_gpsimd_drain=True))

        wave_lo = [sum(WAVES[:k]) for k in range(len(WAVES))]
        wave_of = [k for k, n in enumerate(WAVES) for _ in range(n)]

        @block.sync
        def _(sync):
            # first TE wave goes first (TensorE is the long pole), then the
            # tiny gather-path inputs, then the remaining waves ~3 in flight
            for k, nw in enumerate(WAVES):
                if k >= 3:  # ~3 waves in flight: no fabric bubbles at the
                    # sem-prop + reissue latency between waves
                    sync.wait_ge(s_v[k - 3], 16)
                if k == 1:
                    sync.dma_start(out=idx_sb[:], in_=idx[:]).then_inc(s_ix, 16)
                    sync.dma_start(out=fgb[:], in_=fg[:]).then_inc(s_fb, 16)
                t0 = wave_lo[k]
                sync.dma_start(
                    out=mega[:, t0 : t0 + nw, :, :],
                    in_=src[:, t0 * 2 * W : (t0 + nw) * 2 * W],
                ).then_inc(s_v[k], 16)
            sync.wait_ge(s_red, 1)
            sync.dma_start(out=out[:], in_=red[0:1, 0:1]).then_inc(s_od, 16)

        @block.gpsimd
        def _(gpsimd):
            gpsimd.wait_ge(s_ix, 16)
            for j in range(NGT):
                gpsimd.indirect_dma_start(
                    out=cbuf[:, j, :],
                    out_offset=None,
                    in_=cent[:],
                    in_offset=bass.IndirectOffsetOnAxis(
                        ap=idx_sb[:, j : j + 1], axis=0
                    ),
                ).then_inc(s_g[j], 16)

        @block.tensor
        def _(tensor):
            # p-state warmup on scratch while the first wave streams
            tensor.wait_ge(s_wscr, 1)
            for _ in range(3):
                tensor.matmul(
                    out=ps[:, 7, :],
                    lhsT=wscr[:, :, 0:P],
                    rhs=wscr[:],
                    start=True,
                    stop=True,
                    perf_mode=mybir.MatmulPerfMode.DoubleRow,
                )
            for t in range(NTE):
                if t in wave_lo:
                    tensor.wait_ge(s_v[wave_of[t]], 16)
                if t >= 8:
                    # bank t-8: op0 frees banks 0-1, op1 frees banks 2-4
                    tensor.wait_ge(s_sqa, 1 if t < 10 else 2)
                # DoubleRow: out = I.T @ f_t + G_t.T @ win_t = f_t - c_y
                tensor.matmul(
                    out=ps[:, t % 8, :],
                    lhsT=mega[:, t, :, D:W],
                    rhs=mega[:, t, :, 0:D],
                    start=True,
                    stop=True,
                    perf_mode=mybir.MatmulPerfMode.DoubleRow,
                ).then_inc(s_d, 1)
            # partition fold once all squares are done
            tensor.wait_ge(s_ones, 1)
            tensor.wait_ge(s_sqa, len(SQ_OPS))
            tensor.wait_ge(s_sqd, NGT)
            tensor.matmul(
                out=ps[0:1, 0, 0:NAC],
                lhsT=ones[:],
                rhs=acc[:],
                start=True,
                stop=True,
            ).then_inc(s_mm, 1)

        @block.vector
        def _(vector):
            vector.memset(wscr[:], 0.0).then_inc(s_wscr, 1)
            vector.memset(scr[:], 0.0).then_inc(s_scr, 1)
            vector.memset(ones[:], 1.0).then_inc(s_ones, 1)
            vector.wait_ge(s_fb, 16)
            for j in range(NGT):
                # subtract into bf16, then 2x-mode self-multiply + accum
                vector.wait_ge(s_g[j], 16)
                vector.tensor_tensor(
                    out=dbuf[:, j, :],
                    in0=fgb[:, j, :],
                    in1=cbuf[:, j, :],
                    op=mybir.AluOpType.add,  # centers are pre-negated
                ).then_inc(s_sub, 1)
                vector.wait_ge(s_sub, j + 1)  # drain before reading dbuf
                vector.scalar_tensor_tensor(
                    out=dbuf[:, j, :],
                    in0=dbuf[:, j, :],
                    scalar=1.0,
                    in1=dbuf[:, j, :],
                    op0=mybir.AluOpType.mult,
                    op1=mybir.AluOpType.mult,
                    accum_out=acc[:, len(SQ_OPS) + j : len(SQ_OPS) + j + 1],
                ).then_inc(s_sqd, 1)
            vector.wait_ge(s_mm, 1)
            vector.reduce_sum(
                out=red[:], in_=ps[0:1, 0, 0:NAC], axis=mybir.AxisListType.X
            ).then_inc(s_red, 1)

        @block.scalar
        def _(scalar):
            # ACT: table preload, then Square+accum over PSUM bank ranges
            scalar.wait_ge(s_scr, 1)
            scalar.activation(
                out=scr[:], in_=scr[:], func=mybir.ActivationFunctionType.Square
            )
            for i, (lo, hi) in enumerate(SQ_OPS):
                scalar.wait_ge(s_d, hi)
                b = lo % 8
                scalar.activation(
                    out=ps[:, b : b + (hi - lo), :],
                    in_=ps[:, b : b + (hi - lo), :],
                    func=mybir.ActivationFunctionType.Square,
                    accum_out=acc[:, i : i + 1],
                ).then_inc(s_sqa, 1)

    nc.compile()
    return nc


def _make_in_maps(features, labels, centers):
    import ml_dtypes

    f8 = ml_dtypes.float8_e4m3fn
    cls = np.asarray(labels)[:, -1].astype(np.int64)
    order = np.argsort(cls, kind="stable")
    y = cls[order].reshape(N_CORES, NT, P)
    feats = np.asarray(features, dtype=f8)[order].reshape(N_CORES, NT, P, D)
    cent_neg_full = (-np.asarray(centers, dtype=np.float32)).astype(f8)
    cent_neg = np.zeros((C + P, D), dtype=f8)
    cent_neg[:C] = cent_neg_full
    eye = np.eye(P, dtype=f8)
    in_maps = []
    for i in range(N_CORES):
        pk = np.zeros((P, NTE, 2, W), dtype=f8)
        for t in range(NTE):
            blk = y[i, t]
            a = int(blk.min())
            span = int(blk.max()) - a + 1
            assert span <= P, f"class window span {span} > {P}"
            pk[:, t, 0, 0:D] = feats[i, t]
            pk[:, t, 1, 0:D] = cent_neg[a : a + P]
            pk[:, t, 0, D:W] = eye
            g = np.zeros((P, P), dtype=f8)
            g[blk - a, np.arange(P)] = 1.0
            pk[:, t, 1, D:W] = g
        # gather-path tiles: features [p, j, :], class ids [p, j]
        fgt = np.ascontiguousarray(
            feats[i, NTE:].transpose(1, 0, 2).reshape(P, NGT * D)
        )
        idxt = np.ascontiguousarray(y[i, NTE:].T.astype(np.int32))
        in_maps.append(
            {
                "src": np.ascontiguousarray(pk.reshape(P, NTE * 2 * W)),
                "fg": fgt,
                "idx": idxt,
                "cent": cent_neg_full,
            }
        )
    return in_maps


def _run(features, labels, centers, trace=False):
    from concourse.bass_utils import run_bass_kernel_spmd

    if "nc" not in _cache:
        _cache["nc"] = _build()
    in_maps = _make_in_maps(features, labels, centers)
    res = run_bass_kernel_spmd(
        _cache["nc"], in_maps, list(range(N_CORES)), trace=trace
    )
    total = sum(float(r["out"][0, 0]) for r in res.results)
    return np.float32(total / N), res


def kernel(features, labels, centers):
    out, _ = _run(features, labels, centers, trace=False)
    return out


# revision 16
# speedup vs baseline: 1.0846x; 1.0846x over previous
"""CenterLoss on 8 Trainium2 NeuronCores — v7: sort-by-class, gather-free,
single-stream packed DMA waves.

mean_i clip(||features_i - centers[labels_i,-1]||^2, 1e-12, 1e12) for
features [16384, 512] f32, labels [16384, 2] int, centers [10000, 512] f32.

Design (see v5/v6 history in git-less form):
  - SWDGE gathers cost ~8-10ns/row of serialized Q7 descriptor generation
    (measured: 8.6us per 1024-row dma_gather, plus ~10us library-reload
    stall), so the center gather is reformulated: HOST sorts rows by class
    (a legal choice of row->core assignment; the mean is permutation-
    invariant). Each 128-row tile then spans a contiguous class window of
    <= 128 ids (max 99 observed), and TensorE reconstructs per-row centers
    AND subtracts in ONE fp8 DoubleRow matmul per tile:
        PSUM = I.T @ f_tile + G_t.T @ win_t = f - c_y
    with win_t = f8(-centers)[a_t : a_t+128] (a contiguous slice) and
    G_t[p, i] = 1 iff sorted row i has class a_t + p (host-built one-hot).
  - Everything a tile needs (f 512B + win 512B + [I;G] 256B per partition)
    is packed into ONE DRAM tensor [128, NT, 2, 640] so each DMA wave is a
    single contiguous HWDGE transfer. SDMA drains concurrent queues
    round-robin at equal rates (measured), so waves are issued at most two
    in flight: chunk k completes ~1.4us after chunk k-1 instead of
    everything completing together.
  - Wave sizes (3,3,3,3,2,2) tiles; squares from PSUM: ACT Square+accum
    on tiles [0,3),[3,6),[8,11),[13,16); DVE on [6,8) and [11,13) via
    PSUM->bf16 copy + self-multiply (DVE cannot read PSUM twice in one op
    and DVE pow has no ISA encoding). Clamp dropped (d2 ~ 680, no-op).
  - ones^T @ acc on TensorE folds partitions; reduce_sum -> [1, 1] out.
    Host sums the 8 per-core partials and divides by N.

fp8 e4m3 inputs (host cast; 2.8e-4 rel err vs the 2e-2 gate). HBM traffic
~2.5MB/core. PE p-state is warmed with dummy matmuls while DMAs stream.
"""

import sys

if "/opt/trn_rl_repo" not in sys.path:
    sys.path.insert(0, "/opt/trn_rl_repo")

import numpy as np

N, D, C = 16384, 512, 10000
N_CORES = 8
NS = N // N_CORES  # 2048 rows per core
P = 128
NT = NS // P  # 16 tiles of 128 rows per core
W = D + P  # 640: per-half payload (512 f/win + 128 I/G)
WAVES = [3, 3, 3, 3, 2, 2]  # tiles per DMA wave
assert sum(WAVES) == NT
# square ops: (engine, tile_lo, tile_hi); PSUM bank of tile t is t % 8
SQ_OPS = [
    ("act", 0, 3),
    ("act", 3, 6),
    ("dve", 6, 8),
    ("act", 8, 11),
    ("dve", 11, 13),
    ("act", 13, 16),
]
N_ACT = sum(1 for e, _, _ in SQ_OPS if e == "act")
N_DVE = sum(1 for e, _, _ in SQ_OPS if e == "dve")

_cache = {}


def _build():
    from contextlib import ExitStack

    from concourse import bacc, mybir

    f8 = mybir.dt.float8e4

    nc = bacc.Bacc("TRN2", target_bir_lowering=False, debug=False)
    # packed per tile and half: [f_t | I], [win_t | G_t] (see _make_in_maps)
    src = nc.dram_tensor("src", [P, NT * 2 * W], f8, kind="ExternalInput")
    out = nc.dram_tensor("out", [1, 1], mybir.dt.float32, kind="ExternalOutput")

    with ExitStack() as ctx:
        # [p, tile, half, 640]: [:, t, i, 0:512] = f/win, [:, t, i, 512:640] = I/G
        mega = ctx.enter_context(nc.sbuf_tensor([P, NT, 2, W], f8))
        wscr = ctx.enter_context(nc.sbuf_tensor([P, 2, D], f8))
        acc = ctx.enter_context(nc.sbuf_tensor([P, len(SQ_OPS)], mybir.dt.float32))
        ones = ctx.enter_context(nc.sbuf_tensor([P, 1], mybir.dt.float32))
        scr = ctx.enter_context(nc.sbuf_tensor([P, 1], mybir.dt.float32))
        cscr = ctx.enter_context(nc.sbuf_tensor([P, 2, D], mybir.dt.bfloat16))
        csq = ctx.enter_context(nc.sbuf_tensor([P, 2, D], mybir.dt.bfloat16))
        red = ctx.enter_context(nc.sbuf_tensor([1, 1], mybir.dt.float32))
        ps = ctx.enter_context(nc.psum_tensor([P, 8, D], mybir.dt.float32))
        s_v = [ctx.enter_context(nc.semaphore(f"s_v{k}")) for k in range(len(WAVES))]
        s_ones = ctx.enter_context(nc.semaphore("s_ones"))
        s_scr = ctx.enter_context(nc.semaphore("s_scr"))
        s_wscr = ctx.enter_context(nc.semaphore("s_wscr"))
        s_d = ctx.enter_context(nc.semaphore("s_d"))
        s_sqa = ctx.enter_context(nc.semaphore("s_sqa"))
        s_sqd = ctx.enter_context(nc.semaphore("s_sqd"))
        s_mm = ctx.enter_context(nc.semaphore("s_mm"))
        s_red = ctx.enter_context(nc.semaphore("s_red"))
        s_od = ctx.enter_context(nc.semaphore("s_od"))
        block = ctx.enter_context(nc.Block(no_gpsimd_drain=True))

        wave_lo = [sum(WAVES[:k]) for k in range(len(WAVES))]
        wave_of = [k for k, n in enumerate(WAVES) for _ in range(n)]

        @block.sync
        def _(sync):
            for k, nw in enumerate(WAVES):
                if k >= 2:  # at most ~2 waves in flight
                    sync.wait_ge(s_v[k - 2], 16)
                t0 = wave_lo[k]
                sync.dma_start(
                    out=mega[:, t0 : t0 + nw, :, :],
                    in_=src[:, t0 * 2 * W : (t0 + nw) * 2 * W],
                ).then_inc(s_v[k], 16)
            sync.wait_ge(s_red, 1)
            sync.dma_start(out=out[:], in_=red[0:1, 0:1]).then_inc(s_od, 16)

        @block.tensor
        def _(tensor):
            # p-state warmup on scratch while the first waves stream
            tensor.wait_ge(s_wscr, 1)
            for _ in range(5):
                tensor.matmul(
                    out=ps[:, 7, :],
                    lhsT=wscr[:, :, 0:P],
                    rhs=wscr[:],
                    start=True,
                    stop=True,
                    perf_mode=mybir.MatmulPerfMode.DoubleRow,
                )
            for t in range(NT):
                if t in wave_lo:
                    tensor.wait_ge(s_v[wave_of[t]], 16)
                if t >= 8:
                    # bank t-8: tiles 0-7 are squared by ops covering banks
                    # 0-5 (ACT ops 0,1 + DVE op 6-7); DVE's copy frees 6-7
                    b = t - 8
                    if b < 3:
                        tensor.wait_ge(s_sqa, 1)
                    elif b < 6:
                        tensor.wait_ge(s_sqa, 2)
                    else:
                        tensor.wait_ge(s_sqd, 1)  # first DVE copy done
                # DoubleRow: out = I.T @ f_t + G_t.T @ win_t = f_t - c_y
                tensor.matmul(
                    out=ps[:, t % 8, :],
                    lhsT=mega[:, t, :, D:W],
                    rhs=mega[:, t, :, 0:D],
                    start=True,
                    stop=True,
                    perf_mode=mybir.MatmulPerfMode.DoubleRow,
                ).then_inc(s_d, 1)
            # partition fold once all squares are done
            tensor.wait_ge(s_ones, 1)
            tensor.wait_ge(s_sqa, N_ACT)
            tensor.wait_ge(s_sqd, 2 * N_DVE)
            tensor.matmul(
                out=ps[0:1, 0, 0 : len(SQ_OPS)],
                lhsT=ones[:],
                rhs=acc[:],
                start=True,
                stop=True,
            ).then_inc(s_mm, 1)

        @block.vector
        def _(vector):
            vector.memset(wscr[:], 0.0).then_inc(s_wscr, 1)
            vector.memset(scr[:], 0.0).then_inc(s_scr, 1)
            vector.memset(ones[:], 1.0).then_inc(s_ones, 1)
            nd = 0
            for i, (eng, lo, hi) in enumerate(SQ_OPS):
                if eng != "dve":
                    continue
                # PSUM -> SBUF bf16 copy (frees the banks), then bf16
                # self-multiply with accum (DVE cannot square from PSUM:
                # two PSUM reads per op are illegal, DVE pow has no ISA)
                vector.wait_ge(s_d, hi)
                if nd > 0:
                    # cscr reuse: the prior self-multiply must fully drain
                    vector.wait_ge(s_sqd, 2 * nd)
                b = lo % 8
                vector.tensor_copy(
                    out=cscr[:, 0 : hi - lo, :], in_=ps[:, b : b + (hi - lo), :]
                ).then_inc(s_sqd, 1)
                nd += 1
                vector.wait_ge(s_sqd, 2 * nd - 1)
                vector.scalar_tensor_tensor(
                    out=csq[:, 0 : hi - lo, :],
                    in0=cscr[:, 0 : hi - lo, :],
                    scalar=1.0,
                    in1=cscr[:, 0 : hi - lo, :],
                    op0=mybir.AluOpType.mult,
                    op1=mybir.AluOpType.mult,
                    accum_out=acc[:, i : i + 1],
                ).then_inc(s_sqd, 1)
            vector.wait_ge(s_mm, 1)
            vector.reduce_sum(
                out=red[:], in_=ps[0:1, 0, 0 : len(SQ_OPS)], axis=mybir.AxisListType.X
            ).then_inc(s_red, 1)

        @block.scalar
        def _(scalar):
            # scalar issues no DMAs in v7: its whole stream is the table
            # preload + the ACT square rounds
            scalar.wait_ge(s_scr, 1)
            scalar.activation(
                out=scr[:], in_=scr[:], func=mybir.ActivationFunctionType.Square
            )
            for i, (eng, lo, hi) in enumerate(SQ_OPS):
                if eng != "act":
                    continue
                scalar.wait_ge(s_d, hi)
                b = lo % 8
                scalar.activation(
                    out=ps[:, b : b + (hi - lo), :],
                    in_=ps[:, b : b + (hi - lo), :],
                    func=mybir.ActivationFunctionType.Square,
                    accum_out=acc[:, i : i + 1],
                ).then_inc(s_sqa, 1)

    nc.compile()
    return nc


def _make_in_maps(features, labels, centers):
    import ml_dtypes

    f8 = ml_dtypes.float8_e4m3fn
    cls = np.asarray(labels)[:, -1].astype(np.int64)
    order = np.argsort(cls, kind="stable")
    y = cls[order].reshape(N_CORES, NT, P)
    feats = np.asarray(features, dtype=f8)[order].reshape(N_CORES, NT, P, D)
    cent_neg = np.zeros((C + P, D), dtype=f8)
    cent_neg[:C] = (-np.asarray(centers, dtype=np.float32)).astype(f8)
    eye = np.eye(P, dtype=f8)
    in_maps = []
    for i in range(N_CORES):
        pk = np.zeros((P, NT, 2, W), dtype=f8)
        for t in range(NT):
            blk = y[i, t]
            a = int(blk.min())
            span = int(blk.max()) - a + 1
            assert span <= P, f"class window span {span} > {P}"
            pk[:, t, 0, 0:D] = feats[i, t]  # f tile (partition = row)
            pk[:, t, 1, 0:D] = cent_neg[a : a + P]  # window (partition = class)
            pk[:, t, 0, D:W] = eye
            # G[p, row] = 1 iff blk[row] == a + p
            g = np.zeros((P, P), dtype=f8)
            g[blk - a, np.arange(P)] = 1.0
            pk[:, t, 1, D:W] = g
        in_maps.append({"src": np.ascontiguousarray(pk.reshape(P, NT * 2 * W))})
    return in_maps


def _run(features, labels, centers, trace=False):
    from concourse.bass_utils import run_bass_kernel_spmd

    if "nc" not in _cache:
        _cache["nc"] = _build()
    in_maps = _make_in_maps(features, labels, centers)
    res = run_bass_kernel_spmd(
        _cache["nc"], in_maps, list(range(N_CORES)), trace=trace
    )
    total = sum(float(r["out"][0, 0]) for r in res.results)
    return np.float32(total / N), res


def kernel(features, labels, centers):
    out, _ = _run(features, labels, centers, trace=False)
    return out
